# revision 10
# baseline (speedup 1.0000x reference)
"""Trainium2 Bass kernel for dense_cnn problem.

Math (per batch element n, C=128 channels, H=W=56, G=8):
  t1 = conv_h(x, w1)          5-tap conv over H with full channel mixing
  t3 = dwconv_h(t1, w3)       3-tap depthwise conv over H
  t4[g] = sum_{c,k} x[c, h, w+2k-2] * w4[c,k,g]   (3 width taps, dil 2)
  out[c] = t3[c] * t4[c % 8]

Device strategy (data-parallel, 4 batch elems per core across 8 cores):
  - Fold t3 = w3 (*) w1 (*) x into ONE 7-tap H-conv with combined weights
    wc[ci, f, co]; 4 small correction matmuls fix output rows 0 and 55
    where the fold wrongly includes t1[-1] / t1[56].
  - All matmuls run as fp8e4 (e4m3) DoubleRow pairs at 0.5 cycles/row
    (2x the bf16 column rate).  Accuracy is recovered with a hi/lo
    split: x ships as xh = e4m3(32x) plus xl = e4m3((32x - xh)*16), and
    every weight w as wh = e4m3(s*w) plus wl = e4m3((s*w - wh)*16).
    Each tap is two DR matmuls plus amortized pair corrections:
       DR_a: (wh   | wh/16) x (xh | xl)      [same tap, both x planes]
       DR_b: (wl/16| wl'/16) x (xh | xh')    [adjacent taps share one DR]
    dropping only the wl*xl term (~2^-14 relative).  Simulated end-to-end
    rel err 1.7e-3 -- better than all-bf16 (2.6e-3).
  - PSUM accumulates fp32; psA = 2^16 * t3, psB = 2^15 * t4.  ScalarE
    copies psB -> SBUF with a 2^-31 immediate scale; VectorE multiplies
    psA against it writing fp16; output DMA'd as fp16 and widened on the
    host.  Per chunk the vector engines do only these two 448-elem ops.
  - Head: input DMAs issued finest-first (batch 0 in 4 row slices); 7
    dummy warm-up matmuls trip the PE_HAM clock gate (1.2 -> 2.4 GHz)
    while the DMAs stream.  Tail: last batch elem stores per-chunk.
"""

import sys

sys.path.insert(0, "/opt/trn_rl_repo")

import ml_dtypes
import numpy as np

import concourse.bacc as bacc
import concourse.bass as bass
import concourse.mybir as mybir
import concourse.tile as tile
from concourse import bass_utils

N, C, H, W, G = 32, 128, 56, 56, 8
NCORES = 8
NPC = N // NCORES  # batch elems per core
CH = 8             # H rows per chunk
NCHUNK = H // CH

F32 = mybir.dt.float32
F16 = mybir.dt.float16
BF16 = mybir.dt.bfloat16
F8 = mybir.dt.float8e4
DRM = mybir.MatmulPerfMode.DoubleRow

SX, SXL = 32.0, 16.0     # x scales: xh = e4m3(SX*x), xl = e4m3((SX*x-xh)*SXL)
SW, SWL = 2048.0, 16.0   # folded conv weight scales
SW4, SWL4 = 1024.0, 16.0  # t4 weight scales
DESCALE = 1.0 / (SW * SX * SW4 * SX)   # 2^-31

TRACE = False
TRACE_DIR = None
LAST_EXEC_NS = None
LAST_RESULTS = None

_COMPILED = None


def _enable_trace_hook():
    """The agent image's ``antenv`` lacks ``axon_hooks``, so the boot-time
    NTFF hook registration silently degraded. Recreate the module and
    register the same ctypes-based hook; also skip the bucket upload."""
    import sys as _sys
    import types

    if "antenv.axon_hooks" not in _sys.modules:
        mod = types.ModuleType("antenv.axon_hooks")
        mod._hook = None

        def set_axon_ntff_profile_hook(h):
            mod._hook = h

        def get_axon_ntff_profile_hook():
            return mod._hook

        mod.set_axon_ntff_profile_hook = set_axon_ntff_profile_hook
        mod.get_axon_ntff_profile_hook = get_axon_ntff_profile_hook
        _sys.modules["antenv.axon_hooks"] = mod
        import antenv

        antenv.axon_hooks = mod

    from antenv.axon_hooks import get_axon_ntff_profile_hook as _get

    if _get() is None:
        from trn_agent_boot.trn_boot import _ntff_profile_via_ctypes

        hook = _ntff_profile_via_ctypes("/opt/axon/libaxon_pjrt.so")
        if hook is not None:
            _sys.modules["antenv.axon_hooks"].set_axon_ntff_profile_hook(hook)

    bass_utils.upload_artifacts = lambda tmpdir: f"local:{tmpdir}"


def _plane_pair(xc, plane, r0, nrows, c0, ncols, pstride):
    """AP over xc=[C,2,H,W] viewing 2 k-planes offset by `pstride` elems,
    starting at (plane, r0, c0): shape [C, 2, nrows, ncols]."""
    base = xc[:, plane, r0 : r0 + nrows, c0 : c0 + ncols]
    return bass.AP(
        tensor=base.tensor,
        ap=[[H * W * 2, C], [pstride, 2], [W, nrows], [1, ncols]],
        offset=base.offset,
    )


def _t3_matmuls(c, pa, xc, wfab_t, wfl_t, wcab_t, wcl_t):
    """Folded 7-tap conv chunk as DR pairs: (lhsT, rhs, out, perf) list.
    Output row o reads x row 8c+o+f-3 for tap f."""
    h0 = c * CH
    mms = []

    def rng(f):
        return max(0, 3 - f - h0), min(CH, H + 3 - f - h0)

    # DR_a per tap: (wh, wh/16) x (xh, xl).  f=3 is full for every chunk.
    for f in (3, 0, 1, 2, 4, 5, 6):
        o_lo, o_hi = rng(f)
        if o_lo >= o_hi:
            continue
        r0 = h0 + o_lo + f - 3
        rhs = xc[:, 0:2, r0 : r0 + (o_hi - o_lo), :]
        mms.append((wfab_t[:, f, :, :], rhs, pa[:, o_lo:o_hi, :], DRM))
    # DR_b adjacent-tap pairs on xh with wl/16; partner of f=6 is a zero tap.
    for f in (0, 2, 4, 6):
        lo, hi = rng(f)
        if f < 6:
            lo2, hi2 = rng(f + 1)
            plo, phi = max(lo, lo2), min(hi, hi2)
            # leftover singles outside the pair intersection
            for fe, s_lo, s_hi in ((f, phi, hi), (f + 1, lo2, plo)):
                if s_lo < s_hi:
                    r0 = h0 + s_lo + fe - 3
                    mms.append((
                        wfl_t[:, fe, :],
                        xc[:, 0, r0 : r0 + (s_hi - s_lo), :],
                        pa[:, s_lo:s_hi, :],
                        None,
                    ))
        else:
            plo, phi = lo, hi
        if plo < phi:
            r0 = h0 + plo + f - 3
            rhs = _plane_pair(xc, 0, r0, phi - plo, 0, W, W)
            mms.append((wfl_t[:, f : f + 2, :], rhs, pa[:, plo:phi, :], DRM))
    # fold corrections: rows 0 / 55 wrongly include t1[-1] / t1[56]
    if c == 0 or c == NCHUNK - 1:
        j0 = 0 if c == 0 else 2
        xrow = 0 if c == 0 else 54
        orow = 0 if c == 0 else CH - 1
        for j in range(2):
            mms.append((
                wcab_t[:, j0 + j, :, :],
                xc[:, 0:2, xrow + j : xrow + j + 1, :],
                pa[:, orow : orow + 1, :],
                DRM,
            ))
        mms.append((
            wcl_t[:, j0 : j0 + 2, :],
            _plane_pair(xc, 0, xrow, 1, 0, W, W),
            pa[:, orow : orow + 1, :],
            DRM,
        ))
    return mms


def _t4_matmuls(c, pb, xc, w4ab_t, w4l_t):
    """t4 chunk as DR pairs; taps -2/0/+2 over W, col-clipped."""
    h0 = c * CH
    mms = []
    # DR_a per tap (w4h, w4h/16) x (xh, xl); k index: 0 = -2, 1 = 0, 2 = +2
    mms.append((w4ab_t[:, 1, :, :], xc[:, 0:2, h0 : h0 + CH, :], pb[:], DRM))
    mms.append((
        w4ab_t[:, 0, :, :],
        xc[:, 0:2, h0 : h0 + CH, 0 : W - 2],
        pb[:, :, 2:W],
        DRM,
    ))
    mms.append((
        w4ab_t[:, 2, :, :],
        xc[:, 0:2, h0 : h0 + CH, 2:W],
        pb[:, :, 0 : W - 2],
        DRM,
    ))
    # DR_b: (w4l[0]/16 | w4l[-2]/16) x (xh cols 2: | xh cols 0:)  out cols 2:
    mms.append((
        w4l_t[:, 0:2, :],
        _plane_pair(xc, 0, h0, CH, 2, W - 2, -2),
        pb[:, :, 2:W],
        DRM,
    ))
    # single: tap0 lo-plane at out cols 0:2
    mms.append((w4l_t[:, 0, :], xc[:, 0, h0 : h0 + CH, 0:2], pb[:, :, 0:2], None))
    # DR_b: (w4l[+2]/16 | zero) x (xh cols 2: | junk)  out cols :54
    mms.append((
        w4l_t[:, 2:4, :],
        _plane_pair(xc, 0, h0, CH, 2, W - 2, -2),
        pb[:, :, 0 : W - 2],
        DRM,
    ))
    return mms


def _build():
    nc = bacc.Bacc(
        "TRN2",
        target_bir_lowering=False,
        debug=False,
        enable_asserts=False,
        num_devices=NCORES,
    )

    x_d = nc.dram_tensor("x_s", (NPC, C, 2, H, W), F8, kind="ExternalInput").ap()
    wfab_d = nc.dram_tensor("wfab", (C, 7, 2, C), F8, kind="ExternalInput").ap()
    wfl_d = nc.dram_tensor("wfl", (C, 8, C), F8, kind="ExternalInput").ap()
    wcab_d = nc.dram_tensor("wcab", (C, 4, 2, C), F8, kind="ExternalInput").ap()
    wcl_d = nc.dram_tensor("wcl", (C, 4, C), F8, kind="ExternalInput").ap()
    w4ab_d = nc.dram_tensor("w4ab", (C, 3, 2, C), F8, kind="ExternalInput").ap()
    w4l_d = nc.dram_tensor("w4l", (C, 4, C), F8, kind="ExternalInput").ap()
    out_d = nc.dram_tensor("out", (NPC, C, H, W), F16, kind="ExternalOutput").ap()

    COPY = mybir.ActivationFunctionType.Copy

    with tile.TileContext(nc) as tc:
        with (
            tc.tile_pool(name="wpool", bufs=1) as wpool,
            tc.tile_pool(name="xpool", bufs=1) as xpool,
            tc.tile_pool(name="t4pool", bufs=3) as t4pool,
            tc.tile_pool(name="opool", bufs=3) as opool,
            tc.tile_pool(name="psA", bufs=4, space="PSUM") as papool,
            tc.tile_pool(name="psB", bufs=3, space="PSUM") as pbpool,
            tc.tile_pool(name="psD", bufs=1, space="PSUM") as pdpool,
        ):
            # Dummy matmuls while the first DMAs stream in: PE_HAM ungates
            # the 2.4 GHz clock only after ~3us of sustained activity.
            dmy = wpool.tile([C, 512], BF16)
            nc.vector.memset(dmy[:], 0.0)
            dps = pdpool.tile([C, 512], F32)
            for _ in range(7):
                nc.tensor.matmul(
                    dps[:], lhsT=dmy[:, 0:C], rhs=dmy[:], start=True, stop=True
                )

            wfab_t = wpool.tile([C, 7, 2, C], F8)
            wfl_t = wpool.tile([C, 8, C], F8)
            wcab_t = wpool.tile([C, 4, 2, C], F8)
            wcl_t = wpool.tile([C, 4, C], F8)
            w4ab_t = wpool.tile([C, 3, 2, C], F8)
            w4l_t = wpool.tile([C, 4, C], F8)

            xcs = []
            for n in range(NPC):
                xc = xpool.tile([C, 2, H, W], F8, name=f"xc{n}")
                xcs.append(xc)

            # DMA order: conv weights first (first LDWEIGHTS needs them),
            # then batch 0 x in fine row slices so chunk 0 starts ASAP.
            nc.sync.dma_start(wfab_t[:], wfab_d[:])
            nc.sync.dma_start(wfl_t[:], wfl_d[:])
            nc.sync.dma_start(xcs[0][:, :, 0:14, :], x_d[0, :, :, 0:14, :])
            nc.sync.dma_start(wcab_t[:], wcab_d[:])
            nc.sync.dma_start(wcl_t[:], wcl_d[:])
            nc.sync.dma_start(w4ab_t[:], w4ab_d[:])
            nc.sync.dma_start(w4l_t[:], w4l_d[:])
            nc.sync.dma_start(xcs[0][:, :, 14:28, :], x_d[0, :, :, 14:28, :])
            nc.sync.dma_start(xcs[0][:, :, 28:42, :], x_d[0, :, :, 28:42, :])
            nc.sync.dma_start(xcs[0][:, :, 42:56, :], x_d[0, :, :, 42:56, :])
            for n in range(1, NPC):
                nc.sync.dma_start(xcs[n][:, :, 0:28, :], x_d[n, :, :, 0:28, :])
                nc.sync.dma_start(xcs[n][:, :, 28:56, :], x_d[n, :, :, 28:56, :])

            for n in range(NPC):
                xc = xcs[n]
                last_n = n == NPC - 1
                ot_pair = None

                for c in range(NCHUNK):
                    h0 = c * CH
                    pa = papool.tile([C, CH, W], F32, name="pa")
                    mms = _t3_matmuls(c, pa, xc, wfab_t, wfl_t, wcab_t, wcl_t)
                    for i, (lhsT, rhs, outap, pm) in enumerate(mms):
                        nc.tensor.matmul(
                            outap, lhsT=lhsT, rhs=rhs, perf_mode=pm,
                            start=(i == 0), stop=(i == len(mms) - 1),
                        )
                    pb = pbpool.tile([C, CH, W], F32, name="pb")
                    mmsb = _t4_matmuls(c, pb, xc, w4ab_t, w4l_t)
                    for i, (lhsT, rhs, outap, pm) in enumerate(mmsb):
                        nc.tensor.matmul(
                            outap, lhsT=lhsT, rhs=rhs, perf_mode=pm,
                            start=(i == 0), stop=(i == len(mmsb) - 1),
                        )
                    # psB -> SBUF with the combined 2^-31 descale
                    t4s = t4pool.tile([C, CH, W], F32, name="t4s")
                    nc.scalar.activation(t4s[:], pb[:], COPY, scale=DESCALE)
                    if last_n:
                        ot = opool.tile([C, CH, W], F16, name="ot")
                        nc.vector.tensor_mul(ot[:], pa[:], t4s[:])
                        nc.sync.dma_start(out_d[n, :, h0 : h0 + CH, :], ot[:])
                    else:
                        if c % 2 == 0:
                            ot_pair = opool.tile([C, 2 * CH, W], F16, name="otp")
                        sl = ot_pair[:, (c % 2) * CH : (c % 2 + 1) * CH, :]
                        nc.vector.tensor_mul(sl, pa[:], t4s[:])
                        if c % 2 == 1 or c == NCHUNK - 1:
                            p0 = (c // 2) * 2 * CH
                            rows = (c % 2 + 1) * CH
                            nc.sync.dma_start(
                                out_d[n, :, p0 : p0 + rows, :],
                                ot_pair[:, 0:rows, :],
                            )

    nc.compile()
    return nc


def _get_compiled():
    global _COMPILED
    if _COMPILED is None:
        _COMPILED = _build()
    return _COMPILED


def _q8(a):
    return np.asarray(a, np.float32).astype(ml_dtypes.float8_e4m3)


def _hilo_planes(wt, s, sl):
    """-> (wh, wh/16, wl/16) e4m3 arrays for weight tensor wt."""
    whf = _q8(wt * s).astype(np.float32)
    wl16 = _q8((wt * s - whf) * sl / 16.0)
    return _q8(whf), _q8(whf / 16.0), wl16


def _prep_weights(w1, w3, w4):
    w1c = np.asarray(w1, dtype=np.float64)[:, :, :, 0]  # (co, ci, 5)
    w3c = np.asarray(w3, dtype=np.float64)[:, 0, :, 0]  # (co, 3)
    wc = np.zeros((C, 7, C))                            # (ci, f, co)
    for d in range(3):
        for e in range(5):
            wc[:, d + e, :] += (w1c[:, :, e] * w3c[:, d][:, None]).T
    corr = np.zeros((C, 4, C))
    for j, e in enumerate((3, 4)):
        corr[:, j, :] = -(w1c[:, :, e] * w3c[:, 0][:, None]).T
    for j, e in enumerate((0, 1)):
        corr[:, 2 + j, :] = -(w1c[:, :, e] * w3c[:, 2][:, None]).T
    w4c = np.asarray(w4, dtype=np.float64)[:, :, 0, :]  # (ci, k, g)
    w4b = np.tile(w4c, (1, 1, C // G))                  # (ci, k, 128)

    wh, wh16, wl16 = _hilo_planes(wc, SW, SWL)
    ch, ch16, cl16 = _hilo_planes(corr, SW, SWL)
    w4h, w4h16, w4l16 = _hilo_planes(w4b, SW4, SWL4)

    wfab = np.ascontiguousarray(np.stack([wh, wh16], axis=2))   # (C,7,2,C)
    wfl = np.zeros((C, 8, C), dtype=ml_dtypes.float8_e4m3)
    wfl[:, 0:7, :] = wl16
    wcab = np.ascontiguousarray(np.stack([ch, ch16], axis=2))   # (C,4,2,C)
    wcl = np.ascontiguousarray(cl16)                            # (C,4,C)
    w4ab = np.ascontiguousarray(np.stack([w4h, w4h16], axis=2))  # (C,3,2,C)
    # pair order for DR_b: [k0, k-2, k+2, zero]
    w4l = np.zeros((C, 4, C), dtype=ml_dtypes.float8_e4m3)
    w4l[:, 0, :] = w4l16[:, 1, :]
    w4l[:, 1, :] = w4l16[:, 0, :]
    w4l[:, 2, :] = w4l16[:, 2, :]
    return wfab, wfl, wcab, wcl, w4ab, w4l


def _prep_x(x):
    x32 = np.asarray(x, dtype=np.float32) * SX
    xh = _q8(x32)
    xl = _q8((x32 - xh.astype(np.float32)) * SXL)
    # (N, C, 2, H, W)
    return np.ascontiguousarray(np.stack([xh, xl], axis=2))


def kernel(x, w1, w3, w4):
    global LAST_EXEC_NS, LAST_RESULTS
    nc = _get_compiled()
    xp = _prep_x(x)
    wfab, wfl, wcab, wcl, w4ab, w4l = _prep_weights(w1, w3, w4)

    in_maps = [
        {
            "x_s": np.ascontiguousarray(xp[i * NPC : (i + 1) * NPC]),
            "wfab": wfab,
            "wfl": wfl,
            "wcab": wcab,
            "wcl": wcl,
            "w4ab": w4ab,
            "w4l": w4l,
        }
        for i in range(NCORES)
    ]
    if TRACE:
        _enable_trace_hook()
    res = bass_utils.run_bass_kernel_spmd(
        nc,
        in_maps,
        core_ids=list(range(NCORES)),
        trace=TRACE,
        tmpdir=TRACE_DIR,
    )
    LAST_EXEC_NS = res.exec_time_ns
    LAST_RESULTS = res
    out = np.concatenate(
        [res.results[i]["out"].astype(np.float32) for i in range(NCORES)], axis=0
    )
    return out


# revision 14
# speedup vs baseline: 1.4330x; 1.4330x over previous
"""Trainium2 Bass kernel for dense_cnn problem.

Math (per batch element n, C=128 channels, H=W=56, G=8):
  t1 = conv_h(x, w1)          5-tap conv over H with full channel mixing
  t3 = dwconv_h(t1, w3)       3-tap depthwise conv over H
  t4[g] = sum_{c,k} x[c, h, w+2k-2] * w4[c,k,g]   (3 width taps, dil 2)
  out[c] = t3[c] * t4[c % 8]

Device strategy (data-parallel, 4 batch elems per core across 8 cores):
  - PE does ONLY the dense work: t1 as a 5-tap conv (clipped shifted
    matmuls) and t4 broadcast to 128 channels (3 taps) -> 8 column
    passes per chunk instead of the 10 the folded-7-tap version needs.
  - The 3-tap depthwise conv runs on the otherwise-idle vector engines.
    ScalarE makes two per-partition-scaled copies of psA:
      t1s = w3[c,1] * t1      v = w3[c,0] * t1
    then (TensorScalarPtr is not codegen-supported on GpSimd, plain
    tensor_tensor is):
      acc = v[h-1] + t1s[h]                            (GpSimd ADD)
      t3  = (w3[c,2]/w3[c,1]) * t1s[h+1] + acc         (DVE STT)
    t1s/v have zero pad rows, so no border special cases.
    t3 for chunk c needs t1s row h0+8 from chunk c+1's copy, so the
    dw/multiply/store pipeline runs one chunk behind the PE.
  - Final multiply on DVE reads psB (t4) straight from PSUM and writes
    fp16; output DMA'd as fp16 (half the bytes) and widened on host.
  - Matmuls in bf16 (fp32 matmul lowers to a LOW_HIGH pair at <half
    throughput); accumulation stays fp32 in PSUM.
  - Head: input DMAs issued finest-first (x batch 0 in 4 row-slices) so
    the first chunk's data lands ASAP; 7 dummy warm-up matmuls trip the
    PE_HAM clock gate (1.2 -> 2.4 GHz) while the DMAs stream.
  - Tail: the last batch elem stores per-chunk (not per-pair) to cut
    the post-last-matmul drain.
"""

import sys

sys.path.insert(0, "/opt/trn_rl_repo")

import ml_dtypes
import numpy as np

import concourse.bacc as bacc
import concourse.bass as bass
import concourse.mybir as mybir
import concourse.tile as tile
from concourse import bass_utils

N, C, H, W, G = 32, 128, 56, 56, 8
NCORES = 8
NPC = N // NCORES  # batch elems per core
CH = 8             # H rows per chunk
NCHUNK = H // CH

F32 = mybir.dt.float32
F16 = mybir.dt.float16
BF16 = mybir.dt.bfloat16

TRACE = False
TRACE_DIR = None
LAST_EXEC_NS = None
LAST_RESULTS = None

_COMPILED = None


def _enable_trace_hook():
    """The agent image's ``antenv`` lacks ``axon_hooks``, so the boot-time
    NTFF hook registration silently degraded. Recreate the module and
    register the same ctypes-based hook; also skip the bucket upload."""
    import sys as _sys
    import types

    if "antenv.axon_hooks" not in _sys.modules:
        mod = types.ModuleType("antenv.axon_hooks")
        mod._hook = None

        def set_axon_ntff_profile_hook(h):
            mod._hook = h

        def get_axon_ntff_profile_hook():
            return mod._hook

        mod.set_axon_ntff_profile_hook = set_axon_ntff_profile_hook
        mod.get_axon_ntff_profile_hook = get_axon_ntff_profile_hook
        _sys.modules["antenv.axon_hooks"] = mod
        import antenv

        antenv.axon_hooks = mod

    from antenv.axon_hooks import get_axon_ntff_profile_hook as _get

    if _get() is None:
        from trn_agent_boot.trn_boot import _ntff_profile_via_ctypes

        hook = _ntff_profile_via_ctypes("/opt/axon/libaxon_pjrt.so")
        if hook is not None:
            _sys.modules["antenv.axon_hooks"].set_axon_ntff_profile_hook(hook)

    bass_utils.upload_artifacts = lambda tmpdir: f"local:{tmpdir}"


def _t1_matmuls(h0, nr, pa, xc, wc_t):
    """5-tap H-conv for rows [h0, h0+nr) with clipping at the H borders.
    Output row o reads x row h0+o+e-2 for tap e."""
    mms = []
    # e=2 covers the full chunk always -> emitted first (start=True)
    for e in (2, 0, 1, 3, 4):
        o_lo = max(0, 2 - e - h0)
        o_hi = min(nr, H + 2 - e - h0)
        if o_lo >= o_hi:
            continue
        r0 = h0 + o_lo + e - 2
        r1 = h0 + o_hi + e - 2
        mms.append((wc_t[:, e, :], xc[:, r0:r1, :], pa[:, o_lo:o_hi, :]))
    return mms


def _t4_matmuls(h0, nr, pb, xc, w4_t):
    """t4 chunk: 3 width taps at offsets -2/0/+2, col-clipped at borders."""
    rows = xc[:, h0 : h0 + nr, :]
    return [
        (w4_t[:, 1, :], rows, pb[:, 0:nr, :]),                            # 0
        (w4_t[:, 0, :], xc[:, h0 : h0 + nr, 0 : W - 2], pb[:, 0:nr, 2:W]),   # -2
        (w4_t[:, 2, :], xc[:, h0 : h0 + nr, 2:W], pb[:, 0:nr, 0 : W - 2]),   # +2
    ]


def _build():
    nc = bacc.Bacc(
        "TRN2",
        target_bir_lowering=False,
        debug=False,
        enable_asserts=False,
        num_devices=NCORES,
    )

    x_d = nc.dram_tensor("x_s", (NPC, C, H, W), BF16, kind="ExternalInput").ap()
    wc_d = nc.dram_tensor("wc5", (C, 5, C), BF16, kind="ExternalInput").ap()
    w4_d = nc.dram_tensor("w4b", (C, 3, C), BF16, kind="ExternalInput").ap()
    sc_d = nc.dram_tensor("scal", (C, 3), F32, kind="ExternalInput").ap()
    out_d = nc.dram_tensor("out", (NPC, C, H, W), F16, kind="ExternalOutput").ap()

    mult = mybir.AluOpType.mult
    add = mybir.AluOpType.add
    COPY = mybir.ActivationFunctionType.Copy

    with tile.TileContext(nc) as tc:
        with (
            tc.tile_pool(name="wpool", bufs=1) as wpool,
            tc.tile_pool(name="xpool", bufs=1) as xpool,
            tc.tile_pool(name="t1pool", bufs=2) as t1pool,
            tc.tile_pool(name="accpool", bufs=3) as accpool,
            tc.tile_pool(name="t3pool", bufs=3) as t3pool,
            tc.tile_pool(name="opool", bufs=3) as opool,
            tc.tile_pool(name="psA", bufs=3, space="PSUM") as papool,
            tc.tile_pool(name="psB", bufs=5, space="PSUM") as pbpool,
        ):
            # Dummy matmuls while the first DMAs stream in: PE_HAM ungates
            # the 2.4 GHz clock only after ~3us of sustained activity.
            # Results land in a PSUM bank that is never read.
            dmy = wpool.tile([C, 512], BF16)
            nc.gpsimd.memset(dmy[:], 0.0)
            dps = papool.tile([C, CH, W], F32, name="pa")
            for _ in range(7):
                nc.tensor.matmul(
                    dps[:], lhsT=dmy[:, 0:C], rhs=dmy[:, 0 : CH * W],
                    start=True, stop=True,
                )

            wc_t = wpool.tile([C, 5, C], BF16)
            w4_t = wpool.tile([C, 3, C], BF16)
            sc_t = wpool.tile([C, 3], F32)

            xcs = []
            for n in range(NPC):
                xc = xpool.tile([C, H, W], BF16, name=f"xc{n}")
                xcs.append(xc)

            # DMA order: weights first (needed by the very first LDWEIGHTS),
            # then batch 0 in fine row slices so chunk 0 can start ASAP.
            nc.sync.dma_start(wc_t[:], wc_d[:])
            nc.sync.dma_start(xcs[0][:, 0:14, :], x_d[0, :, 0:14, :])
            nc.sync.dma_start(sc_t[:], sc_d[:])
            nc.sync.dma_start(w4_t[:], w4_d[:])
            nc.sync.dma_start(xcs[0][:, 14:28, :], x_d[0, :, 14:28, :])
            nc.sync.dma_start(xcs[0][:, 28:42, :], x_d[0, :, 28:42, :])
            nc.sync.dma_start(xcs[0][:, 42:56, :], x_d[0, :, 42:56, :])
            for n in range(1, NPC):
                nc.sync.dma_start(xcs[n][:, 0:28, :], x_d[n, :, 0:28, :])
                nc.sync.dma_start(xcs[n][:, 28:56, :], x_d[n, :, 28:56, :])

            w31 = sc_t[:, 0:1]
            w30 = sc_t[:, 1:2]
            r2 = sc_t[:, 2:3]

            for n in range(NPC):
                xc = xcs[n]
                last_n = n == NPC - 1

                # t1s rows: 0 = zero pad (h=-1), 1..56 = h, 57 = zero pad
                t1s = t1pool.tile([C, H + 2, W], BF16, name="t1s")
                nc.gpsimd.memset(t1s[:, 0:1, :], 0.0)
                nc.gpsimd.memset(t1s[:, H + 1 : H + 2, :], 0.0)
                # v rows: 0 = zero pad (h=-1), 1..56 = h
                v = t1pool.tile([C, H + 1, W], BF16, name="v")
                nc.gpsimd.memset(v[:, 0:1, :], 0.0)

                # Last batch elem ends with two 4-row chunks: the post-last-
                # matmul drain is the serial Act->GpSimd->DVE->DMA chain for
                # the final chunk, so a smaller final chunk shortens the tail.
                if last_n:
                    chunks = [(i * CH, CH) for i in range(NCHUNK - 1)]
                    chunks += [(48, 4), (52, 4)]
                else:
                    chunks = [(i * CH, CH) for i in range(NCHUNK)]
                nck = len(chunks)

                accs = [None] * nck
                pbs = [None] * nck
                ots = [None] * nck

                def emit_front(ci):
                    """PE matmuls + Act copies + GpSimd first dw op."""
                    h0, nr = chunks[ci]
                    pa = papool.tile([C, CH, W], F32, name="pa")
                    mms = _t1_matmuls(h0, nr, pa, xc, wc_t)
                    for i, (lhsT, rhs, outap) in enumerate(mms):
                        nc.tensor.matmul(
                            outap, lhsT=lhsT, rhs=rhs,
                            start=(i == 0), stop=(i == len(mms) - 1),
                        )
                    pb = pbpool.tile([C, CH, W], F32, name="pb")
                    for i, (lhsT, rhs, outap) in enumerate(
                        _t4_matmuls(h0, nr, pb, xc, w4_t)
                    ):
                        nc.tensor.matmul(
                            outap, lhsT=lhsT, rhs=rhs,
                            start=(i == 0), stop=(i == 2),
                        )
                    pbs[ci] = pb
                    # t1s[1+h0 : 1+h0+nr] = w3_1 * t1   (per-partition scale)
                    nc.scalar.activation(
                        t1s[:, 1 + h0 : 1 + h0 + nr, :], pa[:, 0:nr, :],
                        COPY, scale=w31,
                    )
                    # v[1+h0 : 1+h0+nr] = w3_0 * t1
                    nc.scalar.activation(
                        v[:, 1 + h0 : 1 + h0 + nr, :], pa[:, 0:nr, :],
                        COPY, scale=w30,
                    )
                    # acc = w3_0*t1[h-1] + w3_1*t1[h]
                    acc = accpool.tile([C, CH, W], BF16, name="acc")
                    nc.gpsimd.tensor_add(
                        acc[:, 0:nr, :],
                        v[:, h0 : h0 + nr, :],
                        t1s[:, 1 + h0 : 1 + h0 + nr, :],
                    )
                    accs[ci] = acc

                def emit_back(ci):
                    """DVE second dw op + final multiply + output DMA.
                    Reads t1s row h0+nr, so chunk ci+1's Act copy must already
                    be emitted (zero pad row for the last chunk)."""
                    h0, nr = chunks[ci]
                    t3 = t3pool.tile([C, CH, W], BF16, name="t3")
                    nc.vector.scalar_tensor_tensor(
                        t3[:, 0:nr, :],
                        t1s[:, 2 + h0 : 2 + h0 + nr, :],
                        r2,
                        accs[ci][:, 0:nr, :],
                        op0=mult, op1=add,
                    )
                    if last_n:
                        ot = opool.tile([C, CH, W], F16, name="ot")
                        nc.vector.tensor_mul(
                            ot[:, 0:nr, :], t3[:, 0:nr, :], pbs[ci][:, 0:nr, :]
                        )
                        nc.sync.dma_start(
                            out_d[n, :, h0 : h0 + nr, :], ot[:, 0:nr, :]
                        )
                        ots[ci] = ot
                    else:
                        c = ci
                        if c % 2 == 0:
                            ots[c] = opool.tile([C, 2 * CH, W], F16, name="otp")
                        else:
                            ots[c] = ots[c - 1]
                        sl = ots[c][:, (c % 2) * CH : (c % 2 + 1) * CH, :]
                        nc.vector.tensor_mul(sl, t3[:], pbs[c][:])
                        if c % 2 == 1 or c == nck - 1:
                            p0 = (c // 2) * 2 * CH
                            rows = (c % 2 + 1) * CH
                            nc.sync.dma_start(
                                out_d[n, :, p0 : p0 + rows, :],
                                ots[c][:, 0:rows, :],
                            )

                for ci in range(nck):
                    emit_front(ci)
                    if ci >= 1:
                        emit_back(ci - 1)
                emit_back(nck - 1)

    nc.compile()
    return nc


def _get_compiled():
    global _COMPILED
    if _COMPILED is None:
        _COMPILED = _build()
    return _COMPILED


def _prep_weights(w1, w3, w4):
    bf = ml_dtypes.bfloat16
    w1c = np.asarray(w1, dtype=np.float32)[:, :, :, 0]  # (co, ci, 5)
    wc5 = np.ascontiguousarray(np.transpose(w1c, (1, 2, 0))).astype(bf)  # (ci,e,co)
    w4c = np.asarray(w4, dtype=np.float32)[:, :, 0, :]  # (ci, k, g)
    w4b = np.ascontiguousarray(np.tile(w4c, (1, 1, C // G))).astype(bf)
    w3c = np.asarray(w3, dtype=np.float32)[:, 0, :, 0]  # (co, 3)
    w31 = w3c[:, 1].copy()
    w31[np.abs(w31) < 1e-12] = 1e-12
    scal = np.stack([w31, w3c[:, 0], w3c[:, 2] / w31], axis=1)
    return wc5, w4b, np.ascontiguousarray(scal, dtype=np.float32)


def kernel(x, w1, w3, w4):
    global LAST_EXEC_NS, LAST_RESULTS
    nc = _get_compiled()
    xb = np.ascontiguousarray(np.asarray(x, dtype=np.float32)).astype(ml_dtypes.bfloat16)
    wc5, w4b, scal = _prep_weights(w1, w3, w4)

    in_maps = [
        {
            "x_s": np.ascontiguousarray(xb[i * NPC : (i + 1) * NPC]),
            "wc5": wc5,
            "w4b": w4b,
            "scal": scal,
        }
        for i in range(NCORES)
    ]
    if TRACE:
        _enable_trace_hook()
    res = bass_utils.run_bass_kernel_spmd(
        nc,
        in_maps,
        core_ids=list(range(NCORES)),
        trace=TRACE,
        tmpdir=TRACE_DIR,
    )
    LAST_EXEC_NS = res.exec_time_ns
    LAST_RESULTS = res
    out = np.concatenate(
        [res.results[i]["out"].astype(np.float32) for i in range(NCORES)], axis=0
    )
    return out


# revision 16
# speedup vs baseline: 1.5860x; 1.1068x over previous
"""Trainium2 Bass kernel for dense_cnn problem.

Math (per batch element n, C=128 channels, H=W=56, G=8):
  t1 = conv_h(x, w1)          5-tap conv over H with full channel mixing
  t3 = dwconv_h(t1, w3)       3-tap depthwise conv over H
  t4[g] = sum_{c,k} x[c, h, w+2k-2] * w4[c,k,g]   (3 width taps, dil 2)
  out[c] = t3[c] * t4[c % 8]

Device strategy (data-parallel, 4 batch elems per core across 8 cores):
  - PE does ONLY the dense work: t1 as a 5-tap conv (clipped shifted
    matmuls) and t4 broadcast to 128 channels (3 taps) -> 8 column
    passes per chunk instead of the 10 the folded-7-tap version needs.
  - The 3-tap depthwise conv runs on the otherwise-idle vector engines.
    ScalarE makes two per-partition-scaled copies of psA:
      t1s = w3[c,1] * t1      v = w3[c,0] * t1
    then (TensorScalarPtr is not codegen-supported on GpSimd, plain
    tensor_tensor is):
      acc = v[h-1] + t1s[h]                            (GpSimd ADD)
      t3  = (w3[c,2]/w3[c,1]) * t1s[h+1] + acc         (DVE STT)
    t1s/v have zero pad rows, so no border special cases.
    t3 for chunk c needs t1s row h0+8 from chunk c+1's copy, so the
    dw/multiply/store pipeline runs one chunk behind the PE.
  - Final multiply on DVE reads psB (t4) straight from PSUM and writes
    fp16; output DMA'd as fp16 (half the bytes) and widened on host.
  - Matmuls in bf16 (fp32 matmul lowers to a LOW_HIGH pair at <half
    throughput); accumulation stays fp32 in PSUM.
  - Head: input DMAs issued finest-first (x batch 0 in 4 row-slices) so
    the first chunk's data lands ASAP; 7 dummy warm-up matmuls trip the
    PE_HAM clock gate (1.2 -> 2.4 GHz) while the DMAs stream.
  - Tail: the last batch elem stores per-chunk (not per-pair) to cut
    the post-last-matmul drain.
"""

import sys

sys.path.insert(0, "/opt/trn_rl_repo")

import ml_dtypes
import numpy as np

import concourse.bacc as bacc
import concourse.bass as bass
import concourse.mybir as mybir
import concourse.tile as tile
from concourse import bass_utils

N, C, H, W, G = 32, 128, 56, 56, 8
NCORES = 8
NPC = N // NCORES  # batch elems per core
CH = 8             # H rows per chunk
NCHUNK = H // CH

F32 = mybir.dt.float32
F16 = mybir.dt.float16
BF16 = mybir.dt.bfloat16

TRACE = False
TRACE_DIR = None
LAST_EXEC_NS = None
LAST_RESULTS = None

_COMPILED = None


def _enable_trace_hook():
    """The agent image's ``antenv`` lacks ``axon_hooks``, so the boot-time
    NTFF hook registration silently degraded. Recreate the module and
    register the same ctypes-based hook; also skip the bucket upload."""
    import sys as _sys
    import types

    if "antenv.axon_hooks" not in _sys.modules:
        mod = types.ModuleType("antenv.axon_hooks")
        mod._hook = None

        def set_axon_ntff_profile_hook(h):
            mod._hook = h

        def get_axon_ntff_profile_hook():
            return mod._hook

        mod.set_axon_ntff_profile_hook = set_axon_ntff_profile_hook
        mod.get_axon_ntff_profile_hook = get_axon_ntff_profile_hook
        _sys.modules["antenv.axon_hooks"] = mod
        import antenv

        antenv.axon_hooks = mod

    from antenv.axon_hooks import get_axon_ntff_profile_hook as _get

    if _get() is None:
        from trn_agent_boot.trn_boot import _ntff_profile_via_ctypes

        hook = _ntff_profile_via_ctypes("/opt/axon/libaxon_pjrt.so")
        if hook is not None:
            _sys.modules["antenv.axon_hooks"].set_axon_ntff_profile_hook(hook)

    bass_utils.upload_artifacts = lambda tmpdir: f"local:{tmpdir}"


def _t1_matmuls(h0, nr, pa, xc, wc_t):
    """5-tap H-conv for rows [h0, h0+nr) with clipping at the H borders.
    Output row o reads x row h0+o+e-2 for tap e."""
    mms = []
    # e=2 covers the full chunk always -> emitted first (start=True)
    for e in (2, 0, 1, 3, 4):
        o_lo = max(0, 2 - e - h0)
        o_hi = min(nr, H + 2 - e - h0)
        if o_lo >= o_hi:
            continue
        r0 = h0 + o_lo + e - 2
        r1 = h0 + o_hi + e - 2
        mms.append((wc_t[:, e, :], xc[:, r0:r1, :], pa[:, o_lo:o_hi, :]))
    return mms


def _t4_matmuls(h0, nr, pb, xc, w4_t):
    """t4 chunk: 3 width taps at offsets -2/0/+2, col-clipped at borders."""
    rows = xc[:, h0 : h0 + nr, :]
    return [
        (w4_t[:, 1, :], rows, pb[:, 0:nr, :]),                            # 0
        (w4_t[:, 0, :], xc[:, h0 : h0 + nr, 0 : W - 2], pb[:, 0:nr, 2:W]),   # -2
        (w4_t[:, 2, :], xc[:, h0 : h0 + nr, 2:W], pb[:, 0:nr, 0 : W - 2]),   # +2
    ]


def _build():
    nc = bacc.Bacc(
        "TRN2",
        target_bir_lowering=False,
        debug=False,
        enable_asserts=False,
        num_devices=NCORES,
    )

    x_d = nc.dram_tensor("x_s", (NPC, C, H, W), BF16, kind="ExternalInput").ap()
    wc_d = nc.dram_tensor("wc5", (C, 5, C), BF16, kind="ExternalInput").ap()
    w4_d = nc.dram_tensor("w4b", (C, 3, C), BF16, kind="ExternalInput").ap()
    sc_d = nc.dram_tensor("scal", (C, 3), F32, kind="ExternalInput").ap()
    out_d = nc.dram_tensor("out", (NPC, C, H, W), F16, kind="ExternalOutput").ap()

    mult = mybir.AluOpType.mult
    add = mybir.AluOpType.add
    COPY = mybir.ActivationFunctionType.Copy

    with tile.TileContext(nc) as tc:
        with (
            tc.tile_pool(name="wpool", bufs=1) as wpool,
            tc.tile_pool(name="xpool", bufs=1) as xpool,
            tc.tile_pool(name="t1pool", bufs=2) as t1pool,
            tc.tile_pool(name="accpool", bufs=3) as accpool,
            tc.tile_pool(name="t3pool", bufs=3) as t3pool,
            tc.tile_pool(name="opool", bufs=3) as opool,
            tc.tile_pool(name="psA", bufs=3, space="PSUM") as papool,
            tc.tile_pool(name="psB", bufs=5, space="PSUM") as pbpool,
        ):
            # Dummy matmuls while the first DMAs stream in: PE_HAM ungates
            # the 2.4 GHz clock only after ~3us of sustained activity.
            # Results land in a PSUM bank that is never read.
            dmy = wpool.tile([C, 512], BF16)
            nc.vector.memset(dmy[:], 0.0)
            dps = papool.tile([C, CH, W], F32, name="pa")
            for _ in range(5):
                nc.tensor.matmul(
                    dps[:], lhsT=dmy[:, 0:C], rhs=dmy[:, 0 : CH * W],
                    start=True, stop=True,
                )

            wc_t = wpool.tile([C, 5, C], BF16)
            w4_t = wpool.tile([C, 3, C], BF16)
            sc_t = wpool.tile([C, 3], F32)

            xcs = []
            for n in range(NPC):
                xc = xpool.tile([C, H, W], BF16, name=f"xc{n}")
                xcs.append(xc)

            # DMA order: weights first (needed by the very first LDWEIGHTS),
            # then batch 0 in fine row slices so chunk 0 can start ASAP.
            nc.sync.dma_start(wc_t[:], wc_d[:])
            nc.sync.dma_start(xcs[0][:, 0:14, :], x_d[0, :, 0:14, :])
            nc.sync.dma_start(xcs[0][:, 14:28, :], x_d[0, :, 14:28, :])
            nc.sync.dma_start(w4_t[:], w4_d[:])
            nc.sync.dma_start(sc_t[:], sc_d[:])
            nc.sync.dma_start(xcs[0][:, 28:42, :], x_d[0, :, 28:42, :])
            nc.sync.dma_start(xcs[0][:, 42:56, :], x_d[0, :, 42:56, :])
            for n in range(1, NPC):
                nc.sync.dma_start(xcs[n][:, 0:28, :], x_d[n, :, 0:28, :])
                nc.sync.dma_start(xcs[n][:, 28:56, :], x_d[n, :, 28:56, :])

            w31 = sc_t[:, 0:1]
            w30 = sc_t[:, 1:2]
            r2 = sc_t[:, 2:3]

            for n in range(NPC):
                xc = xcs[n]
                last_n = n == NPC - 1

                # t1s rows: 0 = zero pad (h=-1), 1..56 = h, 57 = zero pad
                t1s = t1pool.tile([C, H + 2, W], F32, name="t1s")
                nc.gpsimd.memset(t1s[:, 0:1, :], 0.0)
                nc.gpsimd.memset(t1s[:, H + 1 : H + 2, :], 0.0)
                # v rows: 0 = zero pad (h=-1), 1..56 = h
                v = t1pool.tile([C, H + 1, W], F32, name="v")
                nc.gpsimd.memset(v[:, 0:1, :], 0.0)

                # Last batch elem ends with two 4-row chunks: the post-last-
                # matmul drain is the serial Act->GpSimd->DVE->DMA chain for
                # the final chunk, so a smaller final chunk shortens the tail.
                if last_n:
                    chunks = [(i * CH, CH) for i in range(NCHUNK - 1)]
                    chunks += [(48, 4), (52, 4)]
                else:
                    chunks = [(i * CH, CH) for i in range(NCHUNK)]
                nck = len(chunks)

                accs = [None] * nck
                pbs = [None] * nck
                ots = [None] * nck

                def emit_front(ci):
                    """PE matmuls + Act copies + GpSimd first dw op."""
                    h0, nr = chunks[ci]
                    pa = papool.tile([C, CH, W], F32, name="pa")
                    mms = _t1_matmuls(h0, nr, pa, xc, wc_t)
                    for i, (lhsT, rhs, outap) in enumerate(mms):
                        nc.tensor.matmul(
                            outap, lhsT=lhsT, rhs=rhs,
                            start=(i == 0), stop=(i == len(mms) - 1),
                        )
                    pb = pbpool.tile([C, CH, W], F32, name="pb")
                    for i, (lhsT, rhs, outap) in enumerate(
                        _t4_matmuls(h0, nr, pb, xc, w4_t)
                    ):
                        nc.tensor.matmul(
                            outap, lhsT=lhsT, rhs=rhs,
                            start=(i == 0), stop=(i == 2),
                        )
                    pbs[ci] = pb
                    # t1s[1+h0 : 1+h0+nr] = w3_1 * t1   (per-partition scale)
                    nc.scalar.activation(
                        t1s[:, 1 + h0 : 1 + h0 + nr, :], pa[:, 0:nr, :],
                        COPY, scale=w31,
                    )
                    # v[1+h0 : 1+h0+nr] = w3_0 * t1
                    nc.scalar.activation(
                        v[:, 1 + h0 : 1 + h0 + nr, :], pa[:, 0:nr, :],
                        COPY, scale=w30,
                    )
                    # acc = w3_0*t1[h-1] + w3_1*t1[h]
                    acc = accpool.tile([C, CH, W], F32, name="acc")
                    nc.gpsimd.tensor_add(
                        acc[:, 0:nr, :],
                        v[:, h0 : h0 + nr, :],
                        t1s[:, 1 + h0 : 1 + h0 + nr, :],
                    )
                    accs[ci] = acc

                def emit_back(ci):
                    """DVE second dw op + final multiply + output DMA.
                    Reads t1s row h0+nr, so chunk ci+1's Act copy must already
                    be emitted (zero pad row for the last chunk)."""
                    h0, nr = chunks[ci]
                    t3 = t3pool.tile([C, CH, W], F32, name="t3")
                    nc.vector.scalar_tensor_tensor(
                        t3[:, 0:nr, :],
                        t1s[:, 2 + h0 : 2 + h0 + nr, :],
                        r2,
                        accs[ci][:, 0:nr, :],
                        op0=mult, op1=add,
                    )
                    if last_n:
                        ot = opool.tile([C, CH, W], F16, name="ot")
                        nc.vector.tensor_mul(
                            ot[:, 0:nr, :], t3[:, 0:nr, :], pbs[ci][:, 0:nr, :]
                        )
                        nc.sync.dma_start(
                            out_d[n, :, h0 : h0 + nr, :], ot[:, 0:nr, :]
                        )
                        ots[ci] = ot
                    else:
                        c = ci
                        if c % 2 == 0:
                            ots[c] = opool.tile([C, 2 * CH, W], F16, name="otp")
                        else:
                            ots[c] = ots[c - 1]
                        sl = ots[c][:, (c % 2) * CH : (c % 2 + 1) * CH, :]
                        nc.vector.tensor_mul(sl, t3[:], pbs[c][:])
                        if c % 2 == 1 or c == nck - 1:
                            p0 = (c // 2) * 2 * CH
                            rows = (c % 2 + 1) * CH
                            nc.sync.dma_start(
                                out_d[n, :, p0 : p0 + rows, :],
                                ots[c][:, 0:rows, :],
                            )

                for ci in range(nck):
                    emit_front(ci)
                    if ci >= 1:
                        emit_back(ci - 1)
                emit_back(nck - 1)

    nc.compile()
    return nc


def _get_compiled():
    global _COMPILED
    if _COMPILED is None:
        _COMPILED = _build()
    return _COMPILED


def _prep_weights(w1, w3, w4):
    bf = ml_dtypes.bfloat16
    w1c = np.asarray(w1, dtype=np.float32)[:, :, :, 0]  # (co, ci, 5)
    wc5 = np.ascontiguousarray(np.transpose(w1c, (1, 2, 0))).astype(bf)  # (ci,e,co)
    w4c = np.asarray(w4, dtype=np.float32)[:, :, 0, :]  # (ci, k, g)
    w4b = np.ascontiguousarray(np.tile(w4c, (1, 1, C // G))).astype(bf)
    w3c = np.asarray(w3, dtype=np.float32)[:, 0, :, 0]  # (co, 3)
    w31 = w3c[:, 1].copy()
    w31[np.abs(w31) < 1e-12] = 1e-12
    scal = np.stack([w31, w3c[:, 0], w3c[:, 2] / w31], axis=1)
    return wc5, w4b, np.ascontiguousarray(scal, dtype=np.float32)


def kernel(x, w1, w3, w4):
    global LAST_EXEC_NS, LAST_RESULTS
    nc = _get_compiled()
    xb = np.ascontiguousarray(np.asarray(x, dtype=np.float32)).astype(ml_dtypes.bfloat16)
    wc5, w4b, scal = _prep_weights(w1, w3, w4)

    in_maps = [
        {
            "x_s": np.ascontiguousarray(xb[i * NPC : (i + 1) * NPC]),
            "wc5": wc5,
            "w4b": w4b,
            "scal": scal,
        }
        for i in range(NCORES)
    ]
    if TRACE:
        _enable_trace_hook()
    res = bass_utils.run_bass_kernel_spmd(
        nc,
        in_maps,
        core_ids=list(range(NCORES)),
        trace=TRACE,
        tmpdir=TRACE_DIR,
    )
    LAST_EXEC_NS = res.exec_time_ns
    LAST_RESULTS = res
    out = np.concatenate(
        [res.results[i]["out"].astype(np.float32) for i in range(NCORES)], axis=0
    )
    return out
